# revision 14
# baseline (speedup 1.0000x reference)
"""AttLIF Trainium2 kernel: Linear(1024->2048) + temporal-attention gate + IF-neuron scan.

Self-contained: hardcodes shapes B=256, T=64, DIN=1024, DH=2048, 8 NeuronCores,
data-parallel over batch (32 batches/core).

Per core:
  x[bt, h] = dataE[bt, k] @ WE[k, h]     K extended with a ones/bias row
  avg[bt]  = dataE[bt, k] @ w_avg[k]     exact row-mean of x (w_avg = mean_h WE)
  mx[bt]   = max_h x[bt, h]              DVE reduce over 512-wide n-chunks
  score    = sigmoid(W2 @ (relu(W1@avg_b) + relu(W1@mx_b)))   tiny PE matmuls
  scan     : u = v + x*score; s = u>=0.6; v = u*(u<0.6)       DVE, T sequential

The GEMM runs in bf16 with a 3-term hi/lo split (x = d_hi@W_hi + d_hi@W_lo +
d_lo@W_hi, K-concatenated to one K=3073 GEMM) giving ~1.5e-5 relative error --
needed because the spike output is binary and flips near threshold; plain bf16
fails the tolerance, fp32 runs at 1/4 TensorE rate. Can switch to fp32/fp32r
via MODE.

Scan groups of BG batches: x lands in scan layout XS[p = b_l*HH + h_hi,
t*JW + j] via rearranging DMAs; spikes overwrite XS in place and stream out
per batch while the next group's GEMM runs on TensorE.
"""
import os
import sys
from contextlib import ExitStack

import numpy as np

sys.path.insert(0, "/opt/trn_rl_repo")

VTH = 0.6
B, T, DIN, DH = 256, 64, 1024, 2048
NCORES = 8
BS = B // NCORES   # 32
NM = BS * T // 128  # 16 m-tiles per core

MODE = os.environ.get("ATTLIF_MODE", "bf16x3")   # bf16x3 | fp32 | fp32r
BG = int(os.environ.get("ATTLIF_BG", "8"))       # batches per scan group


def _prep_weights(W, bias, W1, W2, mode):
    if mode == "bf16x3":
        import ml_dtypes
        bf = ml_dtypes.bfloat16
        Whi32 = W.astype(bf).astype(np.float32)
        Wlo = (W - Whi32).astype(bf).astype(np.float32)
        WE = np.concatenate([Whi32.T, Wlo.T, Whi32.T, bias[None, :]], axis=0)
        store = bf
    else:
        WE = np.concatenate([W.T, bias[None, :]], axis=0)
        store = np.float32
    KE = WE.shape[0]
    NK = (KE + 127) // 128
    KPAD = NK * 128
    WEp = np.zeros((KPAD, DH), np.float32)
    WEp[:KE] = WE
    wavg = WEp.mean(axis=1)
    wT = np.ascontiguousarray(
        WEp.reshape(NK, 128, DH // 512, 512).transpose(2, 0, 1, 3)).astype(store)
    # [128, NK, 2] column pairs (value, 0) — fp32r needs even moving/out dims
    wavg_arr = np.zeros((128, NK, 2), np.float32)
    wavg_arr[:, :, 0] = wavg.reshape(NK, 128).T
    wavg_arr = np.ascontiguousarray(wavg_arr.reshape(128, 2 * NK)).astype(store)
    w1t = np.ascontiguousarray(W1.T).astype(np.float32)
    w2t = np.ascontiguousarray(W2.T).astype(np.float32)
    return dict(wT=wT, wavg=wavg_arr, w1t=w1t, w2t=w2t), NK, KPAD, store


def _prep_data_shard(shard, mode, NK, KPAD, store):
    rows = shard.reshape(BS * T, DIN).astype(np.float32)
    if mode == "bf16x3":
        import ml_dtypes
        bf = ml_dtypes.bfloat16
        dhi32 = rows.astype(bf).astype(np.float32)
        dlo = (rows - dhi32).astype(bf).astype(np.float32)
        dE = np.concatenate(
            [dhi32, dhi32, dlo, np.ones((BS * T, 1), np.float32)], axis=1)
    else:
        dE = np.concatenate([rows, np.ones((BS * T, 1), np.float32)], axis=1)
    dEp = np.zeros((BS * T, KPAD), np.float32)
    dEp[:, :dE.shape[1]] = dE
    return np.ascontiguousarray(
        dEp.reshape(NM, 128, NK, 128).transpose(0, 2, 3, 1)).astype(store)


def _build(nc, tile, mybir, op_dtype, NK, bg):
    f32 = mybir.dt.float32
    NG = BS // bg
    MG = bg // 2
    JW = (bg * DH) // 128
    HH = DH // JW
    NN = DH // 512
    aop = mybir.AluOpType

    dT = nc.dram_tensor("dT", [NM, NK, 128, 128], op_dtype, kind="ExternalInput").ap()
    wT = nc.dram_tensor("wT", [NN, NK, 128, 512], op_dtype, kind="ExternalInput").ap()
    wavg = nc.dram_tensor("wavg", [128, 2 * NK], op_dtype, kind="ExternalInput").ap()
    w1t = nc.dram_tensor("w1t", [T, 4], f32, kind="ExternalInput").ap()
    w2t = nc.dram_tensor("w2t", [4, T], f32, kind="ExternalInput").ap()
    out = nc.dram_tensor("out", [BS, T, DH], f32, kind="ExternalOutput").ap()

    with tile.TileContext(nc) as tc, ExitStack() as ctx:
        cpool = ctx.enter_context(tc.tile_pool(name="cpool", bufs=1))
        wpool = ctx.enter_context(tc.tile_pool(name="wpool", bufs=1))
        dpool = ctx.enter_context(tc.tile_pool(name="dpool", bufs=bg // 2 + 1))
        xmpool = ctx.enter_context(tc.tile_pool(name="xmpool", bufs=4))
        xspool = ctx.enter_context(tc.tile_pool(name="xspool", bufs=2))
        stpool = ctx.enter_context(tc.tile_pool(name="stpool", bufs=2))
        scpool = ctx.enter_context(tc.tile_pool(name="scpool", bufs=2))
        vpool = ctx.enter_context(tc.tile_pool(name="vpool", bufs=2))
        upool = ctx.enter_context(tc.tile_pool(name="upool", bufs=3))
        pgemm = ctx.enter_context(tc.tile_pool(name="pgemm", bufs=3, space="PSUM"))
        pavg = ctx.enter_context(tc.tile_pool(name="pavg", bufs=2, space="PSUM"))
        pmisc = ctx.enter_context(tc.tile_pool(name="pmisc", bufs=1, space="PSUM"))

        wavg_sb = cpool.tile([128, 2 * NK], op_dtype, name="wavg_sb")
        nc.sync.dma_start(wavg_sb[:], wavg[:])
        w1t_sb = cpool.tile([128, 4], f32, name="w1t_sb")
        nc.sync.dma_start(w1t_sb[0:T, :], w1t[:])
        nc.sync.dma_start(w1t_sb[T:128, :], w1t[:])
        w2t_sb = cpool.tile([4, T], f32, name="w2t_sb")
        nc.sync.dma_start(w2t_sb[:], w2t[:])

        # resident weights: load all n-chunks once
        wcs = []
        for n in range(NN):
            wc = wpool.tile([128, NK * 512], op_dtype, name=f"wc{n}")
            (nc.sync if n % 2 == 0 else nc.scalar).dma_start(
                wc[:], wT[n].rearrange("k kp j -> kp k j"))
            wcs.append(wc)
        _qrr = [0]
        _queues = [nc.sync, nc.scalar, nc.gpsimd]
        def q():
            _qrr[0] += 1
            return _queues[_qrr[0] % 3]
        for g in range(NG):
            XS = xspool.tile([128, T * JW], f32, name="XS", tag="XS")
            stats = stpool.tile([128, 2 * MG], f32, name="stats", tag="stats")
            rmx = stpool.tile([128, NN * MG], f32, name="rmx", tag="rmx")

            # load this group's stationary data tiles once (reused across all n)
            dts = []
            for ml in range(MG):
                dt = dpool.tile([128, NK * 128], op_dtype, name="dt", tag="dt")
                q().dma_start(
                    dt[:], dT[g * MG + ml].rearrange("k kp j -> kp k j"))
                dts.append(dt)
            for n in range(NN):
                wc = wcs[n]
                for ml in range(MG):
                    dt = dts[ml]
                    ps = pgemm.tile([128, 512], f32, name="ps", tag="ps")
                    for k in range(NK):
                        nc.tensor.matmul(ps[:], dt[:, k * 128:(k + 1) * 128],
                                         wc[:, k * 512:(k + 1) * 512],
                                         start=(k == 0), stop=(k == NK - 1))
                    if n == 0:
                        pa = pavg.tile([128, 2], f32, name="pa", tag="pa")
                        for k in range(NK):
                            nc.tensor.matmul(pa[:], dt[:, k * 128:(k + 1) * 128],
                                             wavg_sb[:, 2 * k:2 * k + 2],
                                             start=(k == 0), stop=(k == NK - 1))
                        nc.vector.tensor_copy(stats[:, ml:ml + 1], pa[:, 0:1])
                    xm = xmpool.tile([128, 512], f32, name="xm", tag="xm")
                    nc.scalar.copy(xm[:], ps[:])
                    nc.vector.tensor_reduce(
                        rmx[:, ml * NN + n: ml * NN + n + 1], xm[:],
                        mybir.AxisListType.X, aop.max)
                    # scan layout: p = h_hi*BG + b_l, free = t*JW + j.
                    # Both bh halves in one DMA (dst partitions contiguous),
                    # issued on the otherwise-idle GpSimd queue.
                    nhh = 512 // JW
                    for h2 in range(nhh):
                        p0 = (n * nhh + h2) * bg + 2 * ml
                        q().dma_start(
                            XS[p0:p0 + 2, :],
                            xm[:, h2 * JW:(h2 + 1) * JW])
            for ml in range(MG):
                nc.vector.tensor_reduce(
                    stats[:, MG + ml:MG + ml + 1],
                    rmx[:, ml * NN:(ml + 1) * NN],
                    mybir.AxisListType.X, aop.max)

            h1a = pmisc.tile([4, 2 * MG], f32, name="h1a", tag="pm1")
            nc.tensor.matmul(h1a[:], w1t_sb[0:T, :], stats[0:T, :],
                             start=True, stop=True)
            h1b = pmisc.tile([4, 2 * MG], f32, name="h1b", tag="pm2")
            nc.tensor.matmul(h1b[:], w1t_sb[T:128, :], stats[T:128, :],
                             start=True, stop=True)
            h1r = scpool.tile([4, 4 * MG], f32, name="h1r", tag="h1r")
            nc.scalar.activation(h1r[:, 0:2 * MG], h1a[:],
                                 mybir.ActivationFunctionType.Relu)
            nc.scalar.activation(h1r[:, 2 * MG:4 * MG], h1b[:],
                                 mybir.ActivationFunctionType.Relu)
            # Ht columns in natural batch order b_l = 2*ml + bh
            Ht = scpool.tile([4, 2 * MG], f32, name="Ht", tag="Ht")
            h4 = h1r[:].rearrange("r (b s m) -> r b s m", b=2, s=2)
            nc.vector.tensor_tensor(
                Ht[:].rearrange("r (m b) -> r b m", b=2), h4[:, :, 0], h4[:, :, 1],
                aop.add)
            # score directly in [b_l, t] layout: spT = Ht.T @ W2T
            spT = pmisc.tile([2 * MG, T], f32, name="spT", tag="pm1")
            nc.tensor.matmul(spT[:], Ht[:], w2t_sb[:], start=True, stop=True)
            scb = scpool.tile([2 * MG, T], f32, name="scb", tag="scb")
            nc.scalar.activation(scb[:], spT[:], mybir.ActivationFunctionType.Sigmoid)
            # replicate score rows to every h_hi block: ssc[hh*bg + b_l, t]
            ssc = scpool.tile([128, T], f32, name="ssc", tag="ssc")
            for hh in range(HH):
                q().dma_start(ssc[hh * bg:(hh + 1) * bg, :], scb[:])

            # scan: u_t = x_t*score + v (stored in place over x_t); v = u*(u<VTH)
            v = vpool.tile([128, JW], f32, name="v", tag="v")
            nc.vector.memset(v[:], 0.0)
            for t in range(T):
                xt = XS[:, t * JW:(t + 1) * JW]
                nc.vector.scalar_tensor_tensor(
                    xt, xt, ssc[:, t:t + 1], v[:], op0=aop.mult, op1=aop.add)
                nc.vector.scalar_tensor_tensor(
                    v[:], xt, VTH, xt, op0=aop.is_lt, op1=aop.mult)
            # bulk spike pass (all 128 partitions, one op), then stream out
            half = T * JW // 2
            for piece in range(2):
                nc.vector.tensor_scalar(
                    XS[:, piece * half:(piece + 1) * half],
                    XS[:, piece * half:(piece + 1) * half],
                    VTH, None, op0=aop.is_ge)
            for hh in range(HH):
                q().dma_start(
                    out[g * bg:(g + 1) * bg, :, hh * JW:(hh + 1) * JW],
                    XS[hh * bg:(hh + 1) * bg, :])


_CACHE = {}


def _get_compiled(mode, bg):
    key = (mode, bg)
    if key in _CACHE:
        return _CACHE[key]
    import concourse.tile as tile
    from concourse import bacc, mybir
    dtypes = {"fp32": mybir.dt.float32, "fp32r": mybir.dt.float32r,
              "bf16x3": mybir.dt.bfloat16}
    KE = 3 * DIN + 1 if mode == "bf16x3" else DIN + 1
    NK = (KE + 127) // 128
    nc = bacc.Bacc("TRN2", target_bir_lowering=False, debug=False, num_devices=1)
    _build(nc, tile, mybir, dtypes[mode], NK, bg)
    nc.compile()
    _CACHE[key] = nc
    return nc


def _prep_all(inputs):
    data = np.asarray(inputs["data"], dtype=np.float32)
    W = np.asarray(inputs["W"], dtype=np.float32)
    bias = np.asarray(inputs["bias"], dtype=np.float32)
    W1 = np.asarray(inputs["W1"], dtype=np.float32)
    W2 = np.asarray(inputs["W2"], dtype=np.float32)
    wargs, NK, KPAD, store = _prep_weights(W, bias, W1, W2, MODE)
    in_maps = []
    for c in range(NCORES):
        shard = data[c * BS:(c + 1) * BS]
        in_maps.append({"dT": _prep_data_shard(shard, MODE, NK, KPAD, store),
                        **wargs})
    return in_maps


def _get_compiled_default():
    return _get_compiled(MODE, BG)


def kernel(data, W, bias, W1, W2):
    from concourse.bass_utils import run_bass_kernel_spmd

    data = np.asarray(data, dtype=np.float32)
    W = np.asarray(W, dtype=np.float32)
    bias = np.asarray(bias, dtype=np.float32)
    W1 = np.asarray(W1, dtype=np.float32)
    W2 = np.asarray(W2, dtype=np.float32)

    wargs, NK, KPAD, store = _prep_weights(W, bias, W1, W2, MODE)
    in_maps = []
    for c in range(NCORES):
        shard = data[c * BS:(c + 1) * BS]
        dTc = _prep_data_shard(shard, MODE, NK, KPAD, store)
        in_maps.append({"dT": dTc, **wargs})

    nc = _get_compiled(MODE, BG)
    res = run_bass_kernel_spmd(nc, in_maps, core_ids=list(range(NCORES)))
    outs = [res.results[c]["out"] for c in range(NCORES)]
    return np.concatenate(outs, axis=0)


if __name__ == "__main__":
    rng = np.random.default_rng(0)
    d = rng.standard_normal((B, T, DIN)).astype(np.float32)
    w = (rng.standard_normal((DH, DIN)) / 32.0).astype(np.float32)
    b = np.zeros(DH, np.float32)
    w1 = (rng.standard_normal((4, T)) / 8.0).astype(np.float32)
    w2 = (rng.standard_normal((T, 4)) / 2.0).astype(np.float32)
    o = kernel(d, w, b, w1, w2)
    print(o.shape, o.dtype, o.mean())



# revision 17
# speedup vs baseline: 1.2211x; 1.2211x over previous
"""AttLIF Trainium2 kernel: Linear(1024->2048) + temporal-attention gate + IF-neuron scan.

Self-contained: hardcodes shapes B=256, T=64, DIN=1024, DH=2048, 8 NeuronCores,
data-parallel over batch (32 batches/core).

Per core:
  x[bt, h] = dataE[bt, k] @ WE[k, h]     K extended with a ones/bias row
  avg[bt]  = dataE[bt, k] @ w_avg[k]     exact row-mean of x (w_avg = mean_h WE)
  mx[bt]   = max_h x[bt, h]              DVE reduce over 512-wide n-chunks
  score    = sigmoid(W2 @ (relu(W1@avg_b) + relu(W1@mx_b)))   tiny PE matmuls
  scan     : u = v + x*score; s = u>=0.6; v = u*(u<0.6)       DVE, T sequential

The GEMM runs in bf16 with a 3-term hi/lo split (x = d_hi@W_hi + d_hi@W_lo +
d_lo@W_hi, K-concatenated to one K=3073 GEMM) giving ~1.5e-5 relative error --
needed because the spike output is binary and flips near threshold; plain bf16
fails the tolerance, fp32 runs at 1/4 TensorE rate. Can switch to fp32/fp32r
via MODE.

Scan groups of BG batches: x lands in scan layout XS[p = b_l*HH + h_hi,
t*JW + j] via rearranging DMAs; spikes overwrite XS in place and stream out
per batch while the next group's GEMM runs on TensorE.
"""
import os
import sys
from contextlib import ExitStack

import numpy as np

sys.path.insert(0, "/opt/trn_rl_repo")

VTH = 0.6
B, T, DIN, DH = 256, 64, 1024, 2048
NCORES = 8
BS = B // NCORES   # 32
NM = BS * T // 128  # 16 m-tiles per core

MODE = os.environ.get("ATTLIF_MODE", "bf16x3")   # bf16x3 | fp32 | fp32r
BG = int(os.environ.get("ATTLIF_BG", "8"))       # batches per scan group


def _prep_weights(W, bias, W1, W2, mode):
    if mode == "bf16x3":
        import ml_dtypes
        bf = ml_dtypes.bfloat16
        Whi32 = W.astype(bf).astype(np.float32)
        Wlo = (W - Whi32).astype(bf).astype(np.float32)
        WE = np.concatenate([Whi32.T, Wlo.T, Whi32.T, bias[None, :]], axis=0)
        store = bf
    else:
        WE = np.concatenate([W.T, bias[None, :]], axis=0)
        store = np.float32
    KE = WE.shape[0]
    NK = (KE + 127) // 128
    KPAD = NK * 128
    WEp = np.zeros((KPAD, DH), np.float32)
    WEp[:KE] = WE
    wavg = WEp.mean(axis=1)
    # dedup: for bf16x3 the Whi block (k 16..24) equals k 0..8 - store once
    if mode == "bf16x3":
        NKW = NK - 8
        Wded = np.concatenate([WEp[:2048], WEp[3072:]], axis=0)  # Whi,Wlo + bias
    else:
        NKW = NK
        Wded = WEp
    wT = np.ascontiguousarray(
        Wded.reshape(NKW, 128, DH // 512, 512).transpose(2, 0, 1, 3)).astype(store)
    # [128, NK, 2] column pairs (value, 0) — fp32r needs even moving/out dims
    wavg_arr = np.zeros((128, NK, 2), np.float32)
    wavg_arr[:, :, 0] = wavg.reshape(NK, 128).T
    wavg_arr = np.ascontiguousarray(wavg_arr.reshape(128, 2 * NK)).astype(store)
    w1t = np.ascontiguousarray(W1.T).astype(np.float32)
    w2t = np.ascontiguousarray(W2.T).astype(np.float32)
    return dict(wT=wT, wavg=wavg_arr, w1t=w1t, w2t=w2t), NK, KPAD, store


def _prep_data_shard(shard, mode, NK, KPAD, store):
    rows = shard.reshape(BS * T, DIN).astype(np.float32)
    if mode == "bf16x3":
        import ml_dtypes
        bf = ml_dtypes.bfloat16
        dhi32 = rows.astype(bf).astype(np.float32)
        dlo = (rows - dhi32).astype(bf).astype(np.float32)
        dE = np.concatenate(
            [dhi32, dhi32, dlo, np.ones((BS * T, 1), np.float32)], axis=1)
    else:
        dE = np.concatenate([rows, np.ones((BS * T, 1), np.float32)], axis=1)
    dEp = np.zeros((BS * T, KPAD), np.float32)
    dEp[:, :dE.shape[1]] = dE
    return np.ascontiguousarray(
        dEp.reshape(NM, 128, NK, 128).transpose(0, 2, 3, 1)).astype(store)


def _build(nc, tile, mybir, op_dtype, NK, bg):
    f32 = mybir.dt.float32
    NG = BS // bg
    MG = bg // 2
    JW = (bg * DH) // 128
    HH = DH // JW
    NN = DH // 512
    aop = mybir.AluOpType

    NKW = NK - 8 if NK == 25 else NK   # bf16x3: Whi stored once
    def wcol(k):
        return k if k < 16 else (k - 16 if k < 24 else 16)
    dT = nc.dram_tensor("dT", [NM, NK, 128, 128], op_dtype, kind="ExternalInput").ap()
    wT = nc.dram_tensor("wT", [NN, NKW, 128, 512], op_dtype, kind="ExternalInput").ap()
    wavg = nc.dram_tensor("wavg", [128, 2 * NK], op_dtype, kind="ExternalInput").ap()
    w1t = nc.dram_tensor("w1t", [T, 4], f32, kind="ExternalInput").ap()
    w2t = nc.dram_tensor("w2t", [4, T], f32, kind="ExternalInput").ap()
    out = nc.dram_tensor("out", [BS, T, DH], f32, kind="ExternalOutput").ap()

    with tile.TileContext(nc) as tc, ExitStack() as ctx:
        cpool = ctx.enter_context(tc.tile_pool(name="cpool", bufs=1))
        wpool = ctx.enter_context(tc.tile_pool(name="wpool", bufs=1))
        dpool = ctx.enter_context(tc.tile_pool(name="dpool", bufs=bg // 2 + 2))
        xmpool = ctx.enter_context(tc.tile_pool(name="xmpool", bufs=4))
        xspool = ctx.enter_context(tc.tile_pool(name="xspool", bufs=2))
        stpool = ctx.enter_context(tc.tile_pool(name="stpool", bufs=2))
        scpool = ctx.enter_context(tc.tile_pool(name="scpool", bufs=2))
        vpool = ctx.enter_context(tc.tile_pool(name="vpool", bufs=2))
        upool = ctx.enter_context(tc.tile_pool(name="upool", bufs=3))
        pgemm = ctx.enter_context(tc.tile_pool(name="pgemm", bufs=3, space="PSUM"))
        pavg = ctx.enter_context(tc.tile_pool(name="pavg", bufs=2, space="PSUM"))
        pmisc = ctx.enter_context(tc.tile_pool(name="pmisc", bufs=1, space="PSUM"))

        wavg_sb = cpool.tile([128, 2 * NK], op_dtype, name="wavg_sb")
        nc.sync.dma_start(wavg_sb[:], wavg[:])
        w1t_sb = cpool.tile([128, 4], f32, name="w1t_sb")
        nc.sync.dma_start(w1t_sb[0:T, :], w1t[:])
        nc.sync.dma_start(w1t_sb[T:128, :], w1t[:])
        w2t_sb = cpool.tile([4, T], f32, name="w2t_sb")
        nc.sync.dma_start(w2t_sb[:], w2t[:])

        # resident weights: load all n-chunks once
        wcs = []
        for n in range(NN):
            wc = wpool.tile([128, NKW * 512], op_dtype, name=f"wc{n}")
            (nc.sync if n % 2 == 0 else nc.scalar).dma_start(
                wc[:], wT[n].rearrange("k kp j -> kp k j"))
            wcs.append(wc)
        tq = 0
        tqueues = [nc.sync, nc.scalar, nc.gpsimd]
        for g in range(NG):
            XS = xspool.tile([128, T * JW], f32, name="XS", tag="XS")
            stats = stpool.tile([128, 2 * MG], f32, name="stats", tag="stats")
            rmx = stpool.tile([128, NN * MG], f32, name="rmx", tag="rmx")

            # load this group's stationary data tiles once (reused across all n)
            dts = []
            for ml in range(MG):
                dt = dpool.tile([128, NK * 128], op_dtype, name="dt", tag="dt")
                nc.gpsimd.dma_start(
                    dt[:], dT[g * MG + ml].rearrange("k kp j -> kp k j"))
                dts.append(dt)
            for n in range(NN):
                wc = wcs[n]
                for ml in range(MG):
                    dt = dts[ml]
                    ps = pgemm.tile([128, 512], f32, name="ps", tag="ps")
                    for k in range(NK):
                        kw = wcol(k) if NK == 25 else k
                        nc.tensor.matmul(ps[:], dt[:, k * 128:(k + 1) * 128],
                                         wc[:, kw * 512:(kw + 1) * 512],
                                         start=(k == 0), stop=(k == NK - 1))
                    if n == 0:
                        pa = pavg.tile([128, 2], f32, name="pa", tag="pa")
                        for k in range(NK):
                            nc.tensor.matmul(pa[:], dt[:, k * 128:(k + 1) * 128],
                                             wavg_sb[:, 2 * k:2 * k + 2],
                                             start=(k == 0), stop=(k == NK - 1))
                        nc.vector.tensor_copy(stats[:, ml:ml + 1], pa[:, 0:1])
                    xm = xmpool.tile([128, 512], f32, name="xm", tag="xm")
                    nc.scalar.copy(xm[:], ps[:])
                    nc.vector.tensor_reduce(
                        rmx[:, ml * NN + n: ml * NN + n + 1], xm[:],
                        mybir.AxisListType.X, aop.max)
                    # scan layout: p = h_hi*BG + b_l, free = t*JW + j.
                    # Both bh halves in one DMA (dst partitions contiguous),
                    # issued on the otherwise-idle GpSimd queue.
                    nhh = 512 // JW
                    for h2 in range(nhh):
                        p0 = (n * nhh + h2) * bg + 2 * ml
                        tqueues[tq % 3].dma_start(
                            XS[p0:p0 + 2, :],
                            xm[:, h2 * JW:(h2 + 1) * JW])
                        tq += 1
            for ml in range(MG):
                nc.vector.tensor_reduce(
                    stats[:, MG + ml:MG + ml + 1],
                    rmx[:, ml * NN:(ml + 1) * NN],
                    mybir.AxisListType.X, aop.max)

            h1a = pmisc.tile([4, 2 * MG], f32, name="h1a", tag="pm1")
            nc.tensor.matmul(h1a[:], w1t_sb[0:T, :], stats[0:T, :],
                             start=True, stop=True)
            h1b = pmisc.tile([4, 2 * MG], f32, name="h1b", tag="pm2")
            nc.tensor.matmul(h1b[:], w1t_sb[T:128, :], stats[T:128, :],
                             start=True, stop=True)
            h1r = scpool.tile([4, 4 * MG], f32, name="h1r", tag="h1r")
            nc.scalar.activation(h1r[:, 0:2 * MG], h1a[:],
                                 mybir.ActivationFunctionType.Relu)
            nc.scalar.activation(h1r[:, 2 * MG:4 * MG], h1b[:],
                                 mybir.ActivationFunctionType.Relu)
            # Ht columns in natural batch order b_l = 2*ml + bh
            Ht = scpool.tile([4, 2 * MG], f32, name="Ht", tag="Ht")
            h4 = h1r[:].rearrange("r (b s m) -> r b s m", b=2, s=2)
            nc.vector.tensor_tensor(
                Ht[:].rearrange("r (m b) -> r b m", b=2), h4[:, :, 0], h4[:, :, 1],
                aop.add)
            # score directly in [b_l, t] layout: spT = Ht.T @ W2T
            spT = pmisc.tile([2 * MG, T], f32, name="spT", tag="pm1")
            nc.tensor.matmul(spT[:], Ht[:], w2t_sb[:], start=True, stop=True)
            scb = scpool.tile([2 * MG, T], f32, name="scb", tag="scb")
            nc.scalar.activation(scb[:], spT[:], mybir.ActivationFunctionType.Sigmoid)
            # replicate score rows to every h_hi block: ssc[hh*bg + b_l, t]
            ssc = scpool.tile([128, T], f32, name="ssc", tag="ssc")
            for hh in range(HH):
                nc.gpsimd.dma_start(ssc[hh * bg:(hh + 1) * bg, :], scb[:])

            # scan: u_t = x_t*score + v (stored in place over x_t); v = u*(u<VTH)
            v = vpool.tile([128, JW], f32, name="v", tag="v")
            nc.vector.memset(v[:], 0.0)
            for t in range(T):
                xt = XS[:, t * JW:(t + 1) * JW]
                nc.vector.scalar_tensor_tensor(
                    xt, xt, ssc[:, t:t + 1], v[:], op0=aop.mult, op1=aop.add)
                nc.vector.scalar_tensor_tensor(
                    v[:], xt, VTH, xt, op0=aop.is_lt, op1=aop.mult)
            # bulk spike pass (all 128 partitions, one op), then stream out
            half = T * JW // 2
            for piece in range(2):
                nc.vector.tensor_scalar(
                    XS[:, piece * half:(piece + 1) * half],
                    XS[:, piece * half:(piece + 1) * half],
                    VTH, None, op0=aop.is_ge)
            for hh in range(HH):
                (nc.sync if hh % 2 == 0 else nc.scalar).dma_start(
                    out[g * bg:(g + 1) * bg, :, hh * JW:(hh + 1) * JW],
                    XS[hh * bg:(hh + 1) * bg, :])


_CACHE = {}


def _get_compiled(mode, bg):
    key = (mode, bg)
    if key in _CACHE:
        return _CACHE[key]
    import concourse.tile as tile
    from concourse import bacc, mybir
    dtypes = {"fp32": mybir.dt.float32, "fp32r": mybir.dt.float32r,
              "bf16x3": mybir.dt.bfloat16}
    KE = 3 * DIN + 1 if mode == "bf16x3" else DIN + 1
    NK = (KE + 127) // 128
    nc = bacc.Bacc("TRN2", target_bir_lowering=False, debug=False, num_devices=1)
    _build(nc, tile, mybir, dtypes[mode], NK, bg)
    nc.compile()
    _CACHE[key] = nc
    return nc


def _prep_all(inputs):
    data = np.asarray(inputs["data"], dtype=np.float32)
    W = np.asarray(inputs["W"], dtype=np.float32)
    bias = np.asarray(inputs["bias"], dtype=np.float32)
    W1 = np.asarray(inputs["W1"], dtype=np.float32)
    W2 = np.asarray(inputs["W2"], dtype=np.float32)
    wargs, NK, KPAD, store = _prep_weights(W, bias, W1, W2, MODE)
    in_maps = []
    for c in range(NCORES):
        shard = data[c * BS:(c + 1) * BS]
        in_maps.append({"dT": _prep_data_shard(shard, MODE, NK, KPAD, store),
                        **wargs})
    return in_maps


def _get_compiled_default():
    return _get_compiled(MODE, BG)


def kernel(data, W, bias, W1, W2):
    from concourse.bass_utils import run_bass_kernel_spmd

    data = np.asarray(data, dtype=np.float32)
    W = np.asarray(W, dtype=np.float32)
    bias = np.asarray(bias, dtype=np.float32)
    W1 = np.asarray(W1, dtype=np.float32)
    W2 = np.asarray(W2, dtype=np.float32)

    wargs, NK, KPAD, store = _prep_weights(W, bias, W1, W2, MODE)
    in_maps = []
    for c in range(NCORES):
        shard = data[c * BS:(c + 1) * BS]
        dTc = _prep_data_shard(shard, MODE, NK, KPAD, store)
        in_maps.append({"dT": dTc, **wargs})

    nc = _get_compiled(MODE, BG)
    res = run_bass_kernel_spmd(nc, in_maps, core_ids=list(range(NCORES)))
    outs = [res.results[c]["out"] for c in range(NCORES)]
    return np.concatenate(outs, axis=0)


if __name__ == "__main__":
    rng = np.random.default_rng(0)
    d = rng.standard_normal((B, T, DIN)).astype(np.float32)
    w = (rng.standard_normal((DH, DIN)) / 32.0).astype(np.float32)
    b = np.zeros(DH, np.float32)
    w1 = (rng.standard_normal((4, T)) / 8.0).astype(np.float32)
    w2 = (rng.standard_normal((T, 4)) / 2.0).astype(np.float32)
    o = kernel(d, w, b, w1, w2)
    print(o.shape, o.dtype, o.mean())



# revision 18
# speedup vs baseline: 1.2717x; 1.0414x over previous
"""AttLIF Trainium2 kernel: Linear(1024->2048) + temporal-attention gate + IF-neuron scan.

Self-contained: hardcodes shapes B=256, T=64, DIN=1024, DH=2048, 8 NeuronCores,
data-parallel over batch (32 batches/core).

Per core:
  x[bt, h] = dataE[bt, k] @ WE[k, h]     K extended with a ones/bias row
  avg[bt]  = dataE[bt, k] @ w_avg[k]     exact row-mean of x (w_avg = mean_h WE)
  mx[bt]   = max_h x[bt, h]              DVE reduce over 512-wide n-chunks
  score    = sigmoid(W2 @ (relu(W1@avg_b) + relu(W1@mx_b)))   tiny PE matmuls
  scan     : u = v + x*score; s = u>=0.6; v = u*(u<0.6)       DVE, T sequential

The GEMM runs in bf16 with a 3-term hi/lo split (x = d_hi@W_hi + d_hi@W_lo +
d_lo@W_hi, K-concatenated to one K=3073 GEMM) giving ~1.5e-5 relative error --
needed because the spike output is binary and flips near threshold; plain bf16
fails the tolerance, fp32 runs at 1/4 TensorE rate. Can switch to fp32/fp32r
via MODE.

Scan groups of BG batches: x lands in scan layout XS[p = b_l*HH + h_hi,
t*JW + j] via rearranging DMAs; spikes overwrite XS in place and stream out
per batch while the next group's GEMM runs on TensorE.
"""
import os
import sys
from contextlib import ExitStack

import numpy as np

sys.path.insert(0, "/opt/trn_rl_repo")

VTH = 0.6
B, T, DIN, DH = 256, 64, 1024, 2048
NCORES = 8
BS = B // NCORES   # 32
NM = BS * T // 128  # 16 m-tiles per core

MODE = os.environ.get("ATTLIF_MODE", "bf16x3")   # bf16x3 | fp32 | fp32r
BG = int(os.environ.get("ATTLIF_BG", "8"))       # batches per scan group


def _prep_weights(W, bias, W1, W2, mode):
    if mode == "bf16x3":
        import ml_dtypes
        bf = ml_dtypes.bfloat16
        Whi32 = W.astype(bf).astype(np.float32)
        Wlo = (W - Whi32).astype(bf).astype(np.float32)
        WE = np.concatenate([Whi32.T, Wlo.T, Whi32.T, bias[None, :]], axis=0)
        store = bf
    else:
        WE = np.concatenate([W.T, bias[None, :]], axis=0)
        store = np.float32
    KE = WE.shape[0]
    NK = (KE + 127) // 128
    KPAD = NK * 128
    WEp = np.zeros((KPAD, DH), np.float32)
    WEp[:KE] = WE
    wavg = WEp.mean(axis=1)
    # dedup: for bf16x3 the Whi block (k 16..24) equals k 0..8 - store once
    if mode == "bf16x3":
        NKW = NK - 8
        Wded = np.concatenate([WEp[:2048], WEp[3072:]], axis=0)  # Whi,Wlo + bias
    else:
        NKW = NK
        Wded = WEp
    wT = np.ascontiguousarray(
        Wded.reshape(NKW, 128, DH // 512, 512).transpose(2, 0, 1, 3)).astype(store)
    # [128, NK, 2] column pairs (value, 0) — fp32r needs even moving/out dims
    wavg_arr = np.zeros((128, NK, 2), np.float32)
    wavg_arr[:, :, 0] = wavg.reshape(NK, 128).T
    wavg_arr = np.ascontiguousarray(wavg_arr.reshape(128, 2 * NK)).astype(store)
    w1t = np.ascontiguousarray(W1.T).astype(np.float32)
    w2t = np.ascontiguousarray(W2.T).astype(np.float32)
    return dict(wT=wT, w1t=w1t, w2t=w2t), NK, KPAD, store


def _prep_data_shard(shard, mode, NK, KPAD, store):
    rows = shard.reshape(BS * T, DIN).astype(np.float32)
    if mode == "bf16x3":
        import ml_dtypes
        bf = ml_dtypes.bfloat16
        dhi32 = rows.astype(bf).astype(np.float32)
        dlo = (rows - dhi32).astype(bf).astype(np.float32)
        dE = np.concatenate(
            [dhi32, dhi32, dlo, np.ones((BS * T, 1), np.float32)], axis=1)
    else:
        dE = np.concatenate([rows, np.ones((BS * T, 1), np.float32)], axis=1)
    dEp = np.zeros((BS * T, KPAD), np.float32)
    dEp[:, :dE.shape[1]] = dE
    return np.ascontiguousarray(
        dEp.reshape(NM, 128, NK, 128).transpose(0, 2, 3, 1)).astype(store)


def _build(nc, tile, mybir, op_dtype, NK, bg):
    f32 = mybir.dt.float32
    NG = BS // bg
    MG = bg // 2
    JW = (bg * DH) // 128
    HH = DH // JW
    NN = DH // 512
    aop = mybir.AluOpType

    NKW = NK - 8 if NK == 25 else NK   # bf16x3: Whi stored once
    def wcol(k):
        return k if k < 16 else (k - 16 if k < 24 else 16)
    dT = nc.dram_tensor("dT", [NM, NK, 128, 128], op_dtype, kind="ExternalInput").ap()
    wT = nc.dram_tensor("wT", [NN, NKW, 128, 512], op_dtype, kind="ExternalInput").ap()
    w1t = nc.dram_tensor("w1t", [T, 4], f32, kind="ExternalInput").ap()
    w2t = nc.dram_tensor("w2t", [4, T], f32, kind="ExternalInput").ap()
    out = nc.dram_tensor("out", [BS, T, DH], f32, kind="ExternalOutput").ap()

    with tile.TileContext(nc) as tc, ExitStack() as ctx:
        cpool = ctx.enter_context(tc.tile_pool(name="cpool", bufs=1))
        wpool = ctx.enter_context(tc.tile_pool(name="wpool", bufs=1))
        dpool = ctx.enter_context(tc.tile_pool(name="dpool", bufs=bg // 2 + 2))
        xmpool = ctx.enter_context(tc.tile_pool(name="xmpool", bufs=4))
        xspool = ctx.enter_context(tc.tile_pool(name="xspool", bufs=2))
        stpool = ctx.enter_context(tc.tile_pool(name="stpool", bufs=2))
        scpool = ctx.enter_context(tc.tile_pool(name="scpool", bufs=2))
        vpool = ctx.enter_context(tc.tile_pool(name="vpool", bufs=2))
        upool = ctx.enter_context(tc.tile_pool(name="upool", bufs=3))
        pgemm = ctx.enter_context(tc.tile_pool(name="pgemm", bufs=3, space="PSUM"))
        pmisc = ctx.enter_context(tc.tile_pool(name="pmisc", bufs=1, space="PSUM"))

        w1t_sb = cpool.tile([128, 4], f32, name="w1t_sb")
        nc.sync.dma_start(w1t_sb[0:T, :], w1t[:])
        nc.sync.dma_start(w1t_sb[T:128, :], w1t[:])
        w2t_sb = cpool.tile([4, T], f32, name="w2t_sb")
        nc.sync.dma_start(w2t_sb[:], w2t[:])

        # resident weights: load all n-chunks once
        wcs = []
        for n in range(NN):
            wc = wpool.tile([128, NKW * 512], op_dtype, name=f"wc{n}")
            (nc.sync if n % 2 == 0 else nc.scalar).dma_start(
                wc[:], wT[n].rearrange("k kp j -> kp k j"))
            wcs.append(wc)
        tq = 0
        tqueues = [nc.sync, nc.scalar, nc.gpsimd]
        for g in range(NG):
            XS = xspool.tile([128, T * JW], f32, name="XS", tag="XS")
            stats = stpool.tile([128, 2 * MG], f32, name="stats", tag="stats")
            rmx = stpool.tile([128, NN * MG], f32, name="rmx", tag="rmx")

            # load this group's stationary data tiles once (reused across all n)
            dts = []
            for ml in range(MG):
                dt = dpool.tile([128, NK * 128], op_dtype, name="dt", tag="dt")
                nc.gpsimd.dma_start(
                    dt[:], dT[g * MG + ml].rearrange("k kp j -> kp k j"))
                dts.append(dt)
            asums = []
            for ml in range(MG):
                asums.append(stpool.tile([128, NN], f32, name="asum",
                                         tag=f"asum{ml}"))
            for n in range(NN):
                wc = wcs[n]
                for ml in range(MG):
                    dt = dts[ml]
                    ps = pgemm.tile([128, 512], f32, name="ps", tag="ps")
                    for k in range(NK):
                        kw = wcol(k) if NK == 25 else k
                        nc.tensor.matmul(ps[:], dt[:, k * 128:(k + 1) * 128],
                                         wc[:, kw * 512:(kw + 1) * 512],
                                         start=(k == 0), stop=(k == NK - 1))
                    asum = asums[ml]
                    xm = xmpool.tile([128, 512], f32, name="xm", tag="xm")
                    # avg for free: ACT accumulates the row-sum during the copy
                    nc.scalar.activation(xm[:], ps[:],
                                         mybir.ActivationFunctionType.Copy,
                                         accum_out=asum[:, n:n + 1])
                    nc.vector.tensor_reduce(
                        rmx[:, ml * NN + n: ml * NN + n + 1], xm[:],
                        mybir.AxisListType.X, aop.max)
                    # scan layout: p = h_hi*BG + b_l, free = t*JW + j.
                    # Both bh halves in one DMA (dst partitions contiguous),
                    # issued on the otherwise-idle GpSimd queue.
                    nhh = 512 // JW
                    for h2 in range(nhh):
                        p0 = (n * nhh + h2) * bg + 2 * ml
                        tqueues[tq % 3].dma_start(
                            XS[p0:p0 + 2, :],
                            xm[:, h2 * JW:(h2 + 1) * JW])
                        tq += 1
            for ml in range(MG):
                nc.vector.tensor_reduce(
                    stats[:, ml:ml + 1], asums[ml][:],
                    mybir.AxisListType.X, aop.add)
                nc.vector.tensor_scalar(
                    stats[:, ml:ml + 1], stats[:, ml:ml + 1],
                    1.0 / DH, None, op0=aop.mult)
                nc.vector.tensor_reduce(
                    stats[:, MG + ml:MG + ml + 1],
                    rmx[:, ml * NN:(ml + 1) * NN],
                    mybir.AxisListType.X, aop.max)

            h1a = pmisc.tile([4, 2 * MG], f32, name="h1a", tag="pm1")
            nc.tensor.matmul(h1a[:], w1t_sb[0:T, :], stats[0:T, :],
                             start=True, stop=True)
            h1b = pmisc.tile([4, 2 * MG], f32, name="h1b", tag="pm2")
            nc.tensor.matmul(h1b[:], w1t_sb[T:128, :], stats[T:128, :],
                             start=True, stop=True)
            h1r = scpool.tile([4, 4 * MG], f32, name="h1r", tag="h1r")
            nc.scalar.activation(h1r[:, 0:2 * MG], h1a[:],
                                 mybir.ActivationFunctionType.Relu)
            nc.scalar.activation(h1r[:, 2 * MG:4 * MG], h1b[:],
                                 mybir.ActivationFunctionType.Relu)
            # Ht columns in natural batch order b_l = 2*ml + bh
            Ht = scpool.tile([4, 2 * MG], f32, name="Ht", tag="Ht")
            h4 = h1r[:].rearrange("r (b s m) -> r b s m", b=2, s=2)
            nc.vector.tensor_tensor(
                Ht[:].rearrange("r (m b) -> r b m", b=2), h4[:, :, 0], h4[:, :, 1],
                aop.add)
            # score directly in [b_l, t] layout: spT = Ht.T @ W2T
            spT = pmisc.tile([2 * MG, T], f32, name="spT", tag="pm1")
            nc.tensor.matmul(spT[:], Ht[:], w2t_sb[:], start=True, stop=True)
            scb = scpool.tile([2 * MG, T], f32, name="scb", tag="scb")
            nc.scalar.activation(scb[:], spT[:], mybir.ActivationFunctionType.Sigmoid)
            # replicate score rows to every h_hi block: ssc[hh*bg + b_l, t]
            ssc = scpool.tile([128, T], f32, name="ssc", tag="ssc")
            for hh in range(HH):
                nc.gpsimd.dma_start(ssc[hh * bg:(hh + 1) * bg, :], scb[:])

            # scan: u_t = x_t*score + v (stored in place over x_t); v = u*(u<VTH)
            v = vpool.tile([128, JW], f32, name="v", tag="v")
            nc.vector.memset(v[:], 0.0)
            for t in range(T):
                xt = XS[:, t * JW:(t + 1) * JW]
                nc.vector.scalar_tensor_tensor(
                    xt, xt, ssc[:, t:t + 1], v[:], op0=aop.mult, op1=aop.add)
                nc.vector.scalar_tensor_tensor(
                    v[:], xt, VTH, xt, op0=aop.is_lt, op1=aop.mult)
            # bulk spike pass (all 128 partitions, one op), then stream out
            half = T * JW // 2
            for piece in range(2):
                nc.vector.tensor_scalar(
                    XS[:, piece * half:(piece + 1) * half],
                    XS[:, piece * half:(piece + 1) * half],
                    VTH, None, op0=aop.is_ge)
            for hh in range(HH):
                (nc.sync if hh % 2 == 0 else nc.scalar).dma_start(
                    out[g * bg:(g + 1) * bg, :, hh * JW:(hh + 1) * JW],
                    XS[hh * bg:(hh + 1) * bg, :])


_CACHE = {}


def _get_compiled(mode, bg):
    key = (mode, bg)
    if key in _CACHE:
        return _CACHE[key]
    import concourse.tile as tile
    from concourse import bacc, mybir
    dtypes = {"fp32": mybir.dt.float32, "fp32r": mybir.dt.float32r,
              "bf16x3": mybir.dt.bfloat16}
    KE = 3 * DIN + 1 if mode == "bf16x3" else DIN + 1
    NK = (KE + 127) // 128
    nc = bacc.Bacc("TRN2", target_bir_lowering=False, debug=False, num_devices=1)
    _build(nc, tile, mybir, dtypes[mode], NK, bg)
    nc.compile()
    _CACHE[key] = nc
    return nc


def _prep_all(inputs):
    data = np.asarray(inputs["data"], dtype=np.float32)
    W = np.asarray(inputs["W"], dtype=np.float32)
    bias = np.asarray(inputs["bias"], dtype=np.float32)
    W1 = np.asarray(inputs["W1"], dtype=np.float32)
    W2 = np.asarray(inputs["W2"], dtype=np.float32)
    wargs, NK, KPAD, store = _prep_weights(W, bias, W1, W2, MODE)
    in_maps = []
    for c in range(NCORES):
        shard = data[c * BS:(c + 1) * BS]
        in_maps.append({"dT": _prep_data_shard(shard, MODE, NK, KPAD, store),
                        **wargs})
    return in_maps


def _get_compiled_default():
    return _get_compiled(MODE, BG)


def kernel(data, W, bias, W1, W2):
    from concourse.bass_utils import run_bass_kernel_spmd

    data = np.asarray(data, dtype=np.float32)
    W = np.asarray(W, dtype=np.float32)
    bias = np.asarray(bias, dtype=np.float32)
    W1 = np.asarray(W1, dtype=np.float32)
    W2 = np.asarray(W2, dtype=np.float32)

    wargs, NK, KPAD, store = _prep_weights(W, bias, W1, W2, MODE)
    in_maps = []
    for c in range(NCORES):
        shard = data[c * BS:(c + 1) * BS]
        dTc = _prep_data_shard(shard, MODE, NK, KPAD, store)
        in_maps.append({"dT": dTc, **wargs})

    nc = _get_compiled(MODE, BG)
    res = run_bass_kernel_spmd(nc, in_maps, core_ids=list(range(NCORES)))
    outs = [res.results[c]["out"] for c in range(NCORES)]
    return np.concatenate(outs, axis=0)


if __name__ == "__main__":
    rng = np.random.default_rng(0)
    d = rng.standard_normal((B, T, DIN)).astype(np.float32)
    w = (rng.standard_normal((DH, DIN)) / 32.0).astype(np.float32)
    b = np.zeros(DH, np.float32)
    w1 = (rng.standard_normal((4, T)) / 8.0).astype(np.float32)
    w2 = (rng.standard_normal((T, 4)) / 2.0).astype(np.float32)
    o = kernel(d, w, b, w1, w2)
    print(o.shape, o.dtype, o.mean())

